# revision 34
# baseline (speedup 1.0000x reference)
"""FAVOR causal self-attention (Performer) Trainium2 kernel.

Sharding: 8 cores = 2 (batch) x 4 (head groups of 4 heads). Each core
computes qkv for its heads, runs chunked linear attention (L=128), applies
its slice of the output projection, and returns a partial (T, C) output;
partials are summed on the host (+ b_proj broadcast).

Structure:
  phase 1   qkv projection (PE-dense, chases arriving DMA slabs)
  phase 2   eq/ekt = exp(omega^T {q,k}) m-major, row-tiled head pairs
  phase 2.5 per-chunk precompute, fully pipelined (no serial deps):
            pk = [projk|-nsq] token-major, ekhf = exp(pk + ln 1/16)
            (both heads + fk columns in one activation), vh = [V|1] * fk * c
  phase 3   chunked FAVOR: A/intra/inter/state matmuls + normalize, all
            4 heads batched per vector/scalar op; phase 4 (c_proj tile +
            output DMA) inlined per chunk.

Layout tricks:
  - k stored per head as ktsq_h (128,T): even heads rows 0:64 = kT,
    64:128 = kT^2; ODD heads swapped so the per-pair omega-projection
    matmuls hit disjoint PE row groups and run concurrently. A row-swapped
    const (on2) recovers [projk|-nsq].
  - v stored as (128, 4*65) with a ones column after each head's 64, so the
    intra and state matmuls take a single (tj,65) moving operand.
  - all DMAs are dense [128,N] copies of host-prearranged images; x/wqk
    split into slabs so phase-1 matmuls chase arriving data.
"""
import math
import sys

sys.path.insert(0, "/opt/trn_rl_repo")

import numpy as np

import concourse.bass as bass
import concourse.mybir as mybir
from concourse.tile import TileContext

T, C = 1024, 1024
NH, D, M = 16, 64, 128
L = 128           # chunk length
HPC = 4           # heads per core
NT = T // 128     # 8 token tiles
NK = C // 128     # 8 contraction tiles
F32, F16 = mybir.dt.float32, mybir.dt.float16
LN_SCALE = math.log(1.0 / 16.0)       # folded into the exps
NEG_HALF_LN_M = -0.5 * math.log(M)
VH_SCALE = math.exp(NEG_HALF_LN_M - LN_SCALE)   # vh = [V|1]*fk*VH_SCALE
N_FILL = 40                           # HAM keep-warm filler matmuls

# consts16 column offsets
C16_OM = 0          # omega2 [128,128]
C16_ON = 128        # [omega|0 ; 0|-0.5]  [128,129]
C16_ON2 = 257       # row-swapped variant [128,129]
C16_MK = 386        # causal mask [128,128]
C16_ID = 514        # identity    [128,128]
C16_BV = 642        # bv row      [1,256]
C16_W = 898


def _split_waits(nc):
    """Walrus codegen accepts 1 sync wait per instruction (2 on
    EventSemaphore). Tile can emit more; hoist the excess onto
    EventSemaphore instructions inserted immediately before, same engine."""
    for fn in nc.m.functions:
        for bb in fn.blocks:
            insts = bb.instructions
            i = 0
            while i < len(insts):
                inst = insts[i]
                si = inst.sync_info
                if si is None:
                    i += 1
                    continue
                waits = list(si.on_wait or [])
                cap = 2 if isinstance(inst, mybir.InstEventSemaphore) else 1
                if len(waits) <= cap:
                    i += 1
                    continue
                keep, excess = waits[:cap], waits[cap:]
                new_insts = []
                for j in range(0, len(excess), 2):
                    ev = mybir.InstEventSemaphore(
                        name=nc.get_next_instruction_name(),
                        engine=inst.engine,
                        ins=[],
                        outs=[],
                        sync_info=mybir.SyncInfo(
                            on_wait=excess[j:j + 2], on_update=[]),
                    )
                    nc.register_instruction(ev)
                    new_insts.append(ev)
                inst.sync_info = mybir.SyncInfo(
                    on_wait=keep, on_update=list(si.on_update or []))
                for k, ev in enumerate(new_insts):
                    insts.insert(i + k, ev)
                i += len(new_insts) + 1


def build_bass():
    nc = bass.Bass()

    ximg = nc.dram_tensor("ximg", [128, 8 * 1024], F16, kind="ExternalInput")
    wqkk = nc.dram_tensor("wqkk", [128, NK * 256], F16, kind="ExternalInput")
    wqkq = nc.dram_tensor("wqkq", [128, NK * 256], F16, kind="ExternalInput")
    wvimg = nc.dram_tensor("wvimg", [128, NK * 256], F16, kind="ExternalInput")
    wpimg = nc.dram_tensor("wpimg", [128, 2 * C], F16, kind="ExternalInput")
    consts16 = nc.dram_tensor("consts16", [128, C16_W], F16, kind="ExternalInput")
    consts32 = nc.dram_tensor("consts32", [128, 4], F32, kind="ExternalInput")
    outp = nc.dram_tensor("outp", [T, C], F16, kind="ExternalOutput")

    Exp = mybir.ActivationFunctionType.Exp
    Ident = mybir.ActivationFunctionType.Identity
    Mult = mybir.AluOpType.mult

    with TileContext(nc) as tc:
        with (
            tc.tile_pool(name="big", bufs=1) as big,          # resident data
            tc.tile_pool(name="cpy", bufs=6) as cpy,          # staging tiles
            tc.tile_pool(name="chk", bufs=4) as chk,          # chunk tiles
            tc.tile_pool(name="col", bufs=8) as col,          # small columns
            tc.tile_pool(name="ps", bufs=1, space="PSUM") as ps,
        ):
            def bankA():
                return ps.tile([128, 512], F32, name="bankA", bufs=5)

            # ---- resident tiles ----
            c16 = big.tile([128, C16_W], F16, name="c16")
            c32 = big.tile([128, 4], F32, name="c32")
            xtall = big.tile([128, NK * T], F16, name="xtall")
            wqkk_all = big.tile([128, NK * 256], F16, name="wqkk_all")
            wqkq_all = big.tile([128, NK * 256], F16, name="wqkq_all")
            wvall = big.tile([128, NK * 256], F16, name="wvall")
            wpall = big.tile([128, 2 * C], F16, name="wpall")

            # ---- DMA kicks: dense images, arrival-ordered, 2 HW queues ----
            xt3 = xtall[:, :].rearrange("p (a t) -> p a t", a=NK)

            def xslab(j):
                # slab j: ki pair (2*(j%4), +1), token half j//4
                kp, th = (j % 4) * 2, j // 4
                return (xt3[:, kp:kp + 2, th * 512:(th + 1) * 512],
                        ximg[:, j * 1024:(j + 1) * 1024]
                        .rearrange("p (a t) -> p a t", a=2))

            # Queues allow ~4 outstanding DMAs; packets of concurrent DMAs
            # round-robin, so the critical first transfers go ALONE, one per
            # queue. Scalar gets only 2 kicks so its engine frees up early
            # for phase-1 evictions; sync blocks on its own queue harmlessly.
            for j in (0, 1, 2, 3):
                o, i_ = xslab(j)
                nc.scalar.dma_start(out=o, in_=i_)
            nc.scalar.dma_start(out=wvall[:, :], in_=wvimg[:, :])
            nc.sync.dma_start(out=wqkk_all[:, 0:1024], in_=wqkk[:, 0:1024])
            nc.sync.dma_start(out=wqkk_all[:, 1024:2048], in_=wqkk[:, 1024:2048])
            nc.sync.dma_start(out=c32, in_=consts32[:, :])
            nc.sync.dma_start(out=wqkq_all[:, 0:1024], in_=wqkq[:, 0:1024])
            nc.sync.dma_start(out=wqkq_all[:, 1024:2048], in_=wqkq[:, 1024:2048])
            nc.sync.dma_start(out=c16, in_=consts16[:, :])
            for j in (4, 5, 6, 7):
                o, i_ = xslab(j)
                nc.sync.dma_start(out=o, in_=i_)
            nc.sync.dma_start(out=wpall[:, :], in_=wpimg[:, :])

            om_sb = c16[:, C16_OM:C16_OM + 128]
            on_sb = c16[:, C16_ON:C16_ON + 129]
            on2_sb = c16[:, C16_ON2:C16_ON2 + 129]
            mk_sb = c16[:, C16_MK:C16_MK + 128]
            id_sb = c16[:, C16_ID:C16_ID + 128]
            bv_sb = c16[0:1, C16_BV:C16_BV + HPC * D]
            bqk_sb = [c32[:, mi:mi + 1] for mi in range(4)]

            junk = big.tile([128, 128], F16, name="junk")
            nc.vector.memset(junk[0:1, 0:1], 0.0)   # cheapest possible write
            ones_r = big.tile([1, 128], F16, name="ones_r")
            nc.vector.memset(ones_r, 1.0)
            lnsc_sb = big.tile([128, 1], F32, name="lnsc")
            nc.vector.memset(lnsc_sb, LN_SCALE)
            wfill = big.tile([128, 260], F16, name="wfill")
            nc.vector.memset(wfill, 0.0)

            # ---- PE warm-up fillers (results never read) ----
            wps = ps.tile([128, 512], F32, name="pk", bufs=2)
            for wi in range(N_FILL):
                nc.tensor.matmul(wps[:, 0:128], junk[:, :],
                                 junk[:, :], start=True, stop=True)

            # state bank, pre-zeroed so state matmuls accumulate start=False
            sp3 = [big.tile([128, 4 * (D + 1)], F16, name=f"spair{j}")
                   for j in range(3)]
            ps_s = ps.tile([128, 4 * (D + 1)], F32, name="psS", bufs=1)
            nc.tensor.matmul(ps_s[:, :], wfill[:, 0:128], wfill[:, 0:260],
                             start=True, stop=True, skip_group_check=True)

            xt_sb = [xtall[:, ki * T:(ki + 1) * T] for ki in range(NK)]
            wv_sb = [wvall[:, ki * HPC * D:(ki + 1) * HPC * D]
                     for ki in range(NK)]
            wp_sb = [wpall[:, ci2 * C:(ci2 + 1) * C] for ci2 in range(2)]

            def kblk(ki, j):
                return wqkk_all[:, ki * 256 + j * 128: ki * 256 + (j + 1) * 128]

            def qblk(ki, j):
                return wqkq_all[:, ki * 256 + j * 128: ki * 256 + (j + 1) * 128]

            # ---- persistent intermediates ----
            qt_sb = [big.tile([128, T], F16, name=f"qt{j}") for j in range(2)]
            ktsq_sb = [big.tile([128, T], F16, name=f"ktsq{h}") for h in range(HPC)]
            eq_sb = [big.tile([128, T], F16, name=f"eq{h}") for h in range(HPC)]
            ekt_sb = [big.tile([128, T], F16, name=f"ekt{h}") for h in range(HPC)]
            v_sb = [big.tile([128, HPC * (D + 1)], F16, name=f"v{ti}")
                    for ti in range(NT)]
            # per-chunk precomputed: ekhf blocks [ekh_h0|fk_h0|ekh_h1|fk_h1]
            ekhf = big.tile([128, 16 * 258], F16, name="ekhf")
            vh_all = [big.tile([128, HPC * (D + 1)], F16, name=f"vh{ti}")
                      for ti in range(NT)]
            yt_all = big.tile([128, 2 * T], F16, name="yt_all")

            # ---- phase 1: qkv projection groups ----
            def qk_group(mi, ni):
                tsl = slice(ni * 512, (ni + 1) * 512)
                p_ = bankA()
                for ki in range(NK):
                    nc.tensor.matmul(
                        p_[:, :],
                        kblk(ki, mi - 2) if mi >= 2 else qblk(ki, mi),
                        xt_sb[ki][:, tsl],
                        start=(ki == 0), stop=(ki == NK - 1))
                if mi < 2:
                    nc.vector.tensor_scalar_add(
                        qt_sb[mi][:, tsl], p_[:, :], bqk_sb[mi])
                else:
                    for par in range(2):
                        h = (mi - 2) * 2 + par
                        rs = par * 64          # psum rows holding this head
                        ds = par * 64          # dest rows: k stays in place
                        os = 64 - par * 64     # other rows get k^2
                        nc.scalar.activation(
                            ktsq_sb[h][ds:ds + 64, tsl], p_[rs:rs + 64, :],
                            Ident, bias=bqk_sb[mi][rs:rs + 64, :], scale=1.0)
                        nc.gpsimd.tensor_tensor(
                            ktsq_sb[h][os:os + 64, tsl],
                            ktsq_sb[h][ds:ds + 64, tsl],
                            ktsq_sb[h][ds:ds + 64, tsl], op=Mult)

            # ---- phase 2: exp(omega^T q), exp(omega^T k), row-tiled pairs ----
            def e_q_pair(mi, ni):
                tsl = slice(ni * 512, (ni + 1) * 512)
                banks = []
                for par in range(2):
                    rs = par * 64
                    p_ = bankA()
                    nc.tensor.matmul(p_[:, :], om_sb[rs:rs + 64, :],
                                     qt_sb[mi][rs:rs + 64, tsl],
                                     start=True, stop=True)
                    banks.append(p_)
                for par in range(2):
                    nc.scalar.activation(eq_sb[2 * mi + par][:, tsl],
                                         banks[par][:, :], Exp,
                                         bias=lnsc_sb[:, :], scale=1.0)

            def e_k_pair(pair, ni):
                tsl = slice(ni * 512, (ni + 1) * 512)
                banks = []
                for par in range(2):
                    h, rs = 2 * pair + par, par * 64
                    p_ = bankA()
                    nc.tensor.matmul(p_[:, :], om_sb[rs:rs + 64, :],
                                     ktsq_sb[h][rs:rs + 64, tsl],
                                     start=True, stop=True)
                    banks.append(p_)
                for par in range(2):
                    nc.scalar.activation(ekt_sb[2 * pair + par][:, tsl],
                                         banks[par][:, :], Exp,
                                         bias=lnsc_sb[:, :], scale=1.0)

            def v_group(ti):
                nc.vector.memset(
                    v_sb[ti][:, :].rearrange("p (h c) -> p h c", c=D + 1)
                    [:, :, D:D + 1], 1.0)
                p_ = bankA()
                for ki in range(NK):
                    nc.tensor.matmul(
                        p_[:, 0:HPC * D],
                        xt_sb[ki][:, ti * 128:(ti + 1) * 128],
                        wv_sb[ki][:, :],
                        start=(ki == 0), stop=False)
                nc.tensor.matmul(p_[:, 0:HPC * D], ones_r[:, :], bv_sb[:, :],
                                 start=False, stop=True)
                nc.vector.tensor_copy(
                    v_sb[ti][:, :].rearrange("p (h c) -> p h c", c=D + 1)
                    [:, :, 0:D],
                    p_[:, 0:HPC * D].rearrange("p (h c) -> p h c", c=D))

            # ---- phase 2.5: per-chunk ekh/fk/vh precompute (pipelined) ----
            def chunk_pre(ci, pair):
                h0, h1 = 2 * pair, 2 * pair + 1
                b = pair * NT + ci
                csl = slice(ci * L, (ci + 1) * L)
                pk = ps.tile([128, 512], F32, name="pk", bufs=2)
                nc.tensor.matmul(pk[:, 0:129], ktsq_sb[h0][:, csl],
                                 on_sb[:, :], start=True, stop=True,
                                 skip_group_check=True)
                nc.tensor.matmul(pk[:, 129:258], ktsq_sb[h1][:, csl],
                                 on2_sb[:, :], start=False, stop=True,
                                 skip_group_check=True)
                # exp over [projk|-nsq] for both heads: ekh + fk in one op
                nc.scalar.activation(
                    ekhf[:, b * 258:(b + 1) * 258]
                    .rearrange("p (a c) -> p a c", a=2),
                    pk[:, 0:258].rearrange("p (a c) -> p a c", a=2),
                    Exp, bias=lnsc_sb[:, :], scale=1.0)
                fk0 = ekhf[:, b * 258 + 128:b * 258 + 129]
                fk_b = bass.AP(tensor=fk0.tensor, offset=fk0.offset,
                               ap=[fk0.ap[0], [129, 2], [0, D + 1]])
                nc.vector.scalar_tensor_tensor(
                    vh_all[ci][:, h0 * (D + 1):(h1 + 1) * (D + 1)]
                    .rearrange("p (a c) -> p a c", a=2),
                    v_sb[ci][:, h0 * (D + 1):(h1 + 1) * (D + 1)]
                    .rearrange("p (a c) -> p a c", a=2),
                    VH_SCALE, fk_b, op0=Mult, op1=Mult)

            # ---- phase 3: chunked FAVOR, 3-stage software pipeline ----
            # A(ci): pA matmuls -> atm (vector), state matmuls, spair copy
            # B(ci): inter/intra matmuls into pY -> rc4, ych (vector)
            # C(ci): transposes -> yt copy, c_proj tile, output DMA
            # Emitted as A(c), B(c-1), C(c-2) so every PE op only consumes
            # results produced >= 1 cycle earlier (no PE stalls on vector).
            atm_t = {}
            ych_t = {}
            pyt_t = {}

            def favor_A(ci):
                csl = slice(ci * L, (ci + 1) * L)
                pA = bankA()
                for h in range(HPC):
                    nc.tensor.matmul(pA[:, h * 128:(h + 1) * 128],
                                     ekt_sb[h][:, csl], eq_sb[h][:, csl],
                                     start=(h == 0), stop=True,
                                     skip_group_check=True)
                atm = chk.tile([128, 512], F16, name="atm")
                atm_t[ci] = atm
                mk_b = bass.AP(
                    tensor=mk_sb.tensor, offset=mk_sb.offset,
                    ap=[mk_sb.ap[0], [0, HPC], mk_sb.ap[1]])
                nc.vector.tensor_tensor(
                    atm[:, :].rearrange("p (a c) -> p a c", a=HPC),
                    pA[:, :].rearrange("p (a c) -> p a c", a=HPC),
                    mk_b, op=Mult)
                # state update (bank pre-zeroed: accumulate with start=False)
                for h in range(HPC):
                    b = (h // 2) * NT + ci
                    ssl = h * (D + 1)
                    nc.tensor.matmul(
                        ps_s[:, ssl:ssl + D + 1],
                        ekhf[:, b * 258 + (h % 2) * 129:
                             b * 258 + (h % 2) * 129 + 128],
                        vh_all[ci][:, ssl:ssl + D + 1],
                        start=False, stop=(ci == NT - 1),
                        skip_group_check=True)
                if ci < NT - 1:
                    nc.vector.tensor_copy(sp3[ci % 3][:, :], ps_s[:, :])

            def favor_B(ci):
                csl = slice(ci * L, (ci + 1) * L)
                atm = atm_t.pop(ci)
                pY = bankA()
                for h in range(HPC):
                    ysl = slice(h * (D + 1), (h + 1) * (D + 1))
                    if ci > 0:
                        nc.tensor.matmul(
                            pY[:, ysl], eq_sb[h][:, csl],
                            sp3[(ci - 1) % 3][:, ysl],
                            start=(h == 0), stop=True,
                            skip_group_check=True)
                    nc.tensor.matmul(
                        pY[:, ysl], atm[:, h * 128:(h + 1) * 128],
                        vh_all[ci][:, ysl],
                        start=(ci == 0 and h == 0), stop=True,
                        skip_group_check=True)
                rc4 = col.tile([128, HPC], F32, name="rc4")
                nc.vector.reciprocal(
                    rc4,
                    pY[:, 0:HPC * (D + 1)]
                    .rearrange("p (a c) -> p a c", a=HPC)
                    [:, :, D:D + 1].rearrange("p a c -> p (a c)"))
                ych = chk.tile([128, 256], F16, name="ych")
                ych_t[ci] = ych
                rc_b = bass.AP(
                    tensor=rc4.tensor, offset=rc4.offset,
                    ap=[rc4.ap[0], rc4.ap[1], [0, D]])
                nc.vector.tensor_tensor(
                    ych[:, :].rearrange("p (a c) -> p a c", a=HPC),
                    pY[:, 0:HPC * (D + 1)]
                    .rearrange("p (a c) -> p a c", a=HPC)[:, :, 0:D],
                    rc_b, op=Mult)

            def favor_C1(ci):
                ych = ych_t.pop(ci)
                pyt = ps.tile([128, 256], F16, name="bankA", bufs=5)
                nc.tensor.matmul(pyt[:, 0:128], ych[:, 0:128], id_sb[:, :],
                                 is_transpose=True, start=True, stop=True,
                                 skip_group_check=True)
                nc.tensor.matmul(pyt[:, 128:256], ych[:, 128:256],
                                 id_sb[:, :], is_transpose=True,
                                 start=False, stop=True,
                                 skip_group_check=True)
                nc.vector.tensor_copy(
                    yt_all[:, :].rearrange("p (a t) -> p a t", a=2)
                    [:, :, ci * 128:(ci + 1) * 128],
                    pyt[:, :].rearrange("p (a c) -> p a c", a=2))

            def favor_C2(ci):
                # ---- phase 4 for this token tile ----
                osb = cpy.tile([128, 1024], F16, name="osb")
                for ni in range(2):
                    nsl = slice(ni * 512, (ni + 1) * 512)
                    pp = bankA()
                    for ci2 in range(2):
                        nc.tensor.matmul(
                            pp[:, :],
                            yt_all[:, ci2 * T + ci * 128:
                                   ci2 * T + (ci + 1) * 128],
                            wp_sb[ci2][:, nsl],
                            start=(ci2 == 0), stop=(ci2 == 1))
                    if ni == 1 and ci == NT - 1:
                        nc.vector.tensor_copy(osb[:, nsl], pp[:, :])
                    else:
                        nc.scalar.copy(osb[:, nsl], pp[:, :])
                    nc.sync.dma_start(
                        out=outp[ci * 128:(ci + 1) * 128, nsl],
                        in_=osb[:, nsl])

            # ---- program order ----
            for ni in range(2):
                qk_group(2, ni)
                qk_group(3, ni)
                e_k_pair(0, ni)
                e_k_pair(1, ni)
                for ti in range(4 * ni, 4 * ni + 4):
                    v_group(ti)
                qk_group(0, ni)
                qk_group(1, ni)
                e_q_pair(0, ni)
                e_q_pair(1, ni)
                for ci in range(4 * ni, 4 * ni + 4):
                    chunk_pre(ci, 0)
                    chunk_pre(ci, 1)
            for c in range(NT):
                favor_A(c)
                if c >= 1:
                    favor_B(c - 1)
                if c >= 2:
                    favor_C1(c - 2)
                if c >= 3:
                    favor_C2(c - 3)
            favor_B(NT - 1)
            favor_C1(NT - 2)
            favor_C2(NT - 3)
            favor_C1(NT - 1)
            favor_C2(NT - 2)
            favor_C2(NT - 1)

    _split_waits(nc)
    return nc


_NC_CACHE = None


def _get_nc():
    global _NC_CACHE
    if _NC_CACHE is None:
        _NC_CACHE = build_bass()
    return _NC_CACHE


def _img8(w):
    # [1024, n] -> [128, 8*n] with 128-row blocks laid side by side
    n = w.shape[1]
    return np.ascontiguousarray(
        w.reshape(8, 128, n).transpose(1, 0, 2).reshape(128, 8 * n))


def kernel(x, W_attn, b_attn, W_proj, b_proj, omega):
    from concourse.bass_utils import run_bass_kernel_spmd

    x = np.asarray(x, dtype=np.float32)
    W_attn = np.asarray(W_attn, dtype=np.float32)
    b_attn = np.asarray(b_attn, dtype=np.float32)
    W_proj = np.asarray(W_proj, dtype=np.float32)
    b_proj = np.asarray(b_proj, dtype=np.float32)
    omega = np.asarray(omega, dtype=np.float32)

    B = x.shape[0]
    scale = 1.0 / math.sqrt(D)
    omega2 = np.concatenate([omega, omega], axis=0)
    omnsq = np.zeros((128, 129), np.float32)
    omnsq[0:64, 0:128] = omega
    omnsq[64:128, 128] = -0.5
    omnsq2 = np.zeros((128, 129), np.float32)
    omnsq2[64:128, 0:128] = omega
    omnsq2[0:64, 128] = -0.5
    maskT = np.triu(np.ones((128, 128), np.float32))
    ident = np.eye(128, dtype=np.float32)
    c16_base = np.zeros((128, C16_W), np.float16)
    c16_base[:, C16_OM:C16_OM + 128] = omega2
    c16_base[:, C16_ON:C16_ON + 129] = omnsq
    c16_base[:, C16_ON2:C16_ON2 + 129] = omnsq2
    c16_base[:, C16_MK:C16_MK + 128] = maskT
    c16_base[:, C16_ID:C16_ID + 128] = ident

    # x images: 8 slabs of (ki pair, token half), token half 0 first
    ximgs = []
    for b in range(B):
        xT = np.ascontiguousarray(x[b].T).astype(np.float16)
        Xr = xT.reshape(8, 128, 1024)
        slabs = []
        for j in range(8):
            kp, th = (j % 4) * 2, j // 4
            s = Xr[kp:kp + 2, :, th * 512:(th + 1) * 512]   # [2,128,512]
            slabs.append(s.transpose(1, 0, 2).reshape(128, 1024))
        ximgs.append(np.ascontiguousarray(np.concatenate(slabs, axis=1)))

    in_maps = []
    for core in range(8):
        b, g = core // 4, core % 4
        ch0 = g * HPC * D
        wq = (W_attn[:, ch0:ch0 + HPC * D] * scale).astype(np.float16)
        wk = (W_attn[:, C + ch0:C + ch0 + HPC * D] * scale).astype(np.float16)
        wv_ = W_attn[:, 2 * C + ch0:2 * C + ch0 + HPC * D].astype(np.float16)
        wp_ = W_proj[ch0:ch0 + HPC * D, :].astype(np.float16)
        bqk_ = (np.concatenate([b_attn[ch0:ch0 + HPC * D],
                                b_attn[C + ch0:C + ch0 + HPC * D]]) * scale
                ).astype(np.float32)
        bv_ = b_attn[2 * C + ch0:2 * C + ch0 + HPC * D].astype(np.float16)
        c16 = c16_base.copy()
        c16[0, C16_BV:C16_BV + HPC * D] = bv_
        c32 = bqk_.reshape(4, 128).T.astype(np.float32)
        in_maps.append({
            "ximg": ximgs[b],
            "wqkk": _img8(wk), "wqkq": _img8(wq), "wvimg": _img8(wv_),
            "wpimg": np.ascontiguousarray(
                wp_.reshape(2, 128, 1024).transpose(1, 0, 2)
                .reshape(128, 2048)),
            "consts16": c16, "consts32": np.ascontiguousarray(c32),
        })

    nc = _get_nc()
    res = run_bass_kernel_spmd(nc, in_maps, list(range(8)))

    out = np.zeros((B, T, C), dtype=np.float32)
    for core in range(8):
        out[core // 4] += res.results[core]["outp"]
    out += b_proj[None, None, :]
    return out


# revision 35
# speedup vs baseline: 1.0782x; 1.0782x over previous
"""FAVOR causal self-attention (Performer) Trainium2 kernel.

Sharding: 8 cores = 2 (batch) x 4 (head groups of 4 heads). Each core
computes qkv for its heads, runs chunked linear attention (L=128), applies
its slice of the output projection, and returns a partial (T, C) output;
partials are summed on the host (+ b_proj broadcast).

Structure:
  phase 1   qkv projection (PE-dense, chases arriving DMA slabs)
  phase 2   eq/ekt = exp(omega^T {q,k}) m-major, row-tiled head pairs
  phase 2.5 per-chunk precompute, fully pipelined (no serial deps):
            pk = [projk|-nsq] token-major, ekhf = exp(pk + ln 1/16)
            (both heads + fk columns in one activation), vh = [V|1] * fk * c
  phase 3   chunked FAVOR: A/intra/inter/state matmuls + normalize, all
            4 heads batched per vector/scalar op; phase 4 (c_proj tile +
            output DMA) inlined per chunk.

Layout tricks:
  - k stored per head as ktsq_h (128,T): even heads rows 0:64 = kT,
    64:128 = kT^2; ODD heads swapped so the per-pair omega-projection
    matmuls hit disjoint PE row groups and run concurrently. A row-swapped
    const (on2) recovers [projk|-nsq].
  - v stored as (128, 4*65) with a ones column after each head's 64, so the
    intra and state matmuls take a single (tj,65) moving operand.
  - all DMAs are dense [128,N] copies of host-prearranged images; x/wqk
    split into slabs so phase-1 matmuls chase arriving data.
"""
import math
import sys

sys.path.insert(0, "/opt/trn_rl_repo")

import numpy as np

import concourse.bass as bass
import concourse.mybir as mybir
from concourse.tile import TileContext

T, C = 1024, 1024
NH, D, M = 16, 64, 128
L = 128           # chunk length
HPC = 4           # heads per core
NT = T // 128     # 8 token tiles
NK = C // 128     # 8 contraction tiles
F32, F16 = mybir.dt.float32, mybir.dt.float16
LN_SCALE = math.log(1.0 / 16.0)       # folded into the exps
NEG_HALF_LN_M = -0.5 * math.log(M)
VH_SCALE = math.exp(NEG_HALF_LN_M - LN_SCALE)   # vh = [V|1]*fk*VH_SCALE
N_FILL = 48                           # HAM keep-warm filler matmuls

# consts16 column offsets
C16_OM = 0          # omega2 [128,128]
C16_ON = 128        # [omega|0 ; 0|-0.5]  [128,129]
C16_ON2 = 257       # row-swapped variant [128,129]
C16_MK = 386        # causal mask [128,128]
C16_ID = 514        # identity    [128,128]
C16_BV = 642        # bv row      [1,256]
C16_W = 898


def _split_waits(nc):
    """Walrus codegen accepts 1 sync wait per instruction (2 on
    EventSemaphore). Tile can emit more; hoist the excess onto
    EventSemaphore instructions inserted immediately before, same engine."""
    for fn in nc.m.functions:
        for bb in fn.blocks:
            insts = bb.instructions
            i = 0
            while i < len(insts):
                inst = insts[i]
                si = inst.sync_info
                if si is None:
                    i += 1
                    continue
                waits = list(si.on_wait or [])
                cap = 2 if isinstance(inst, mybir.InstEventSemaphore) else 1
                if len(waits) <= cap:
                    i += 1
                    continue
                keep, excess = waits[:cap], waits[cap:]
                new_insts = []
                for j in range(0, len(excess), 2):
                    ev = mybir.InstEventSemaphore(
                        name=nc.get_next_instruction_name(),
                        engine=inst.engine,
                        ins=[],
                        outs=[],
                        sync_info=mybir.SyncInfo(
                            on_wait=excess[j:j + 2], on_update=[]),
                    )
                    nc.register_instruction(ev)
                    new_insts.append(ev)
                inst.sync_info = mybir.SyncInfo(
                    on_wait=keep, on_update=list(si.on_update or []))
                for k, ev in enumerate(new_insts):
                    insts.insert(i + k, ev)
                i += len(new_insts) + 1


def build_bass():
    nc = bass.Bass()

    ximg = nc.dram_tensor("ximg", [128, 8 * 1024], F16, kind="ExternalInput")
    wqkk = nc.dram_tensor("wqkk", [128, NK * 256], F16, kind="ExternalInput")
    wqkq = nc.dram_tensor("wqkq", [128, NK * 256], F16, kind="ExternalInput")
    wvimg = nc.dram_tensor("wvimg", [128, NK * 256], F16, kind="ExternalInput")
    wpimg = nc.dram_tensor("wpimg", [128, 2 * C], F16, kind="ExternalInput")
    consts16 = nc.dram_tensor("consts16", [128, C16_W], F16, kind="ExternalInput")
    consts32 = nc.dram_tensor("consts32", [128, 4], F32, kind="ExternalInput")
    outp = nc.dram_tensor("outp", [T, C], F16, kind="ExternalOutput")

    Exp = mybir.ActivationFunctionType.Exp
    Ident = mybir.ActivationFunctionType.Identity
    Mult = mybir.AluOpType.mult

    with TileContext(nc) as tc:
        with (
            tc.tile_pool(name="big", bufs=1) as big,          # resident data
            tc.tile_pool(name="cpy", bufs=6) as cpy,          # staging tiles
            tc.tile_pool(name="chk", bufs=4) as chk,          # chunk tiles
            tc.tile_pool(name="col", bufs=8) as col,          # small columns
            tc.tile_pool(name="ps", bufs=1, space="PSUM") as ps,
        ):
            def bankA():
                return ps.tile([128, 512], F32, name="bankA", bufs=5)

            # ---- resident tiles ----
            c16 = big.tile([128, C16_W], F16, name="c16")
            c32 = big.tile([128, 4], F32, name="c32")
            xtall = big.tile([128, NK * T], F16, name="xtall")
            wqkk_all = big.tile([128, NK * 256], F16, name="wqkk_all")
            wqkq_all = big.tile([128, NK * 256], F16, name="wqkq_all")
            wvall = big.tile([128, NK * 256], F16, name="wvall")
            wpall = big.tile([128, 2 * C], F16, name="wpall")

            # ---- DMA kicks: dense images, arrival-ordered, 2 HW queues ----
            xt3 = xtall[:, :].rearrange("p (a t) -> p a t", a=NK)

            def xslab(j):
                # slab j: ki pair (2*(j%4), +1), token half j//4
                kp, th = (j % 4) * 2, j // 4
                return (xt3[:, kp:kp + 2, th * 512:(th + 1) * 512],
                        ximg[:, j * 1024:(j + 1) * 1024]
                        .rearrange("p (a t) -> p a t", a=2))

            # Queues allow ~4 outstanding DMAs; packets of concurrent DMAs
            # round-robin, so the critical first transfers go ALONE, one per
            # queue. Scalar gets only 2 kicks so its engine frees up early
            # for phase-1 evictions; sync blocks on its own queue harmlessly.
            for j in (0, 1, 2, 3):
                o, i_ = xslab(j)
                nc.scalar.dma_start(out=o, in_=i_)
            nc.scalar.dma_start(out=wvall[:, :], in_=wvimg[:, :])
            nc.sync.dma_start(out=wqkk_all[:, 0:1024], in_=wqkk[:, 0:1024])
            nc.sync.dma_start(out=wqkk_all[:, 1024:2048], in_=wqkk[:, 1024:2048])
            nc.sync.dma_start(out=c32, in_=consts32[:, :])
            nc.sync.dma_start(out=c16, in_=consts16[:, :])
            nc.sync.dma_start(out=wqkq_all[:, 0:1024], in_=wqkq[:, 0:1024])
            nc.sync.dma_start(out=wqkq_all[:, 1024:2048], in_=wqkq[:, 1024:2048])
            for j in (4, 5, 6, 7):
                o, i_ = xslab(j)
                nc.sync.dma_start(out=o, in_=i_)
            nc.sync.dma_start(out=wpall[:, :], in_=wpimg[:, :])

            om_sb = c16[:, C16_OM:C16_OM + 128]
            on_sb = c16[:, C16_ON:C16_ON + 129]
            on2_sb = c16[:, C16_ON2:C16_ON2 + 129]
            mk_sb = c16[:, C16_MK:C16_MK + 128]
            id_sb = c16[:, C16_ID:C16_ID + 128]
            bv_sb = c16[0:1, C16_BV:C16_BV + HPC * D]
            bqk_sb = [c32[:, mi:mi + 1] for mi in range(4)]

            junk = big.tile([128, 128], F16, name="junk")
            nc.vector.memset(junk[0:1, 0:1], 0.0)   # cheapest possible write
            ones_r = big.tile([1, 128], F16, name="ones_r")
            nc.vector.memset(ones_r, 1.0)
            lnsc_sb = big.tile([128, 1], F32, name="lnsc")
            nc.vector.memset(lnsc_sb, LN_SCALE)
            wfill = big.tile([128, 260], F16, name="wfill")
            nc.vector.memset(wfill, 0.0)

            # ---- PE warm-up fillers (results never read) ----
            wps = ps.tile([128, 512], F32, name="pk", bufs=2)
            for wi in range(N_FILL):
                nc.tensor.matmul(wps[:, 0:128], junk[:, :],
                                 junk[:, :], start=True, stop=True)

            # state bank, pre-zeroed so state matmuls accumulate start=False
            sp3 = [big.tile([128, 4 * (D + 1)], F16, name=f"spair{j}")
                   for j in range(3)]
            ps_s = ps.tile([128, 4 * (D + 1)], F32, name="psS", bufs=1)
            nc.tensor.matmul(ps_s[:, :], wfill[:, 0:128], wfill[:, 0:260],
                             start=True, stop=True, skip_group_check=True)

            xt_sb = [xtall[:, ki * T:(ki + 1) * T] for ki in range(NK)]
            wv_sb = [wvall[:, ki * HPC * D:(ki + 1) * HPC * D]
                     for ki in range(NK)]
            wp_sb = [wpall[:, ci2 * C:(ci2 + 1) * C] for ci2 in range(2)]

            def kblk(ki, j):
                return wqkk_all[:, ki * 256 + j * 128: ki * 256 + (j + 1) * 128]

            def qblk(ki, j):
                return wqkq_all[:, ki * 256 + j * 128: ki * 256 + (j + 1) * 128]

            # ---- persistent intermediates ----
            qt_sb = [big.tile([128, T], F16, name=f"qt{j}") for j in range(2)]
            ktsq_sb = [big.tile([128, T], F16, name=f"ktsq{h}") for h in range(HPC)]
            eq_sb = [big.tile([128, T], F16, name=f"eq{h}") for h in range(HPC)]
            ekt_sb = [big.tile([128, T], F16, name=f"ekt{h}") for h in range(HPC)]
            v_sb = [big.tile([128, HPC * (D + 1)], F16, name=f"v{ti}")
                    for ti in range(NT)]
            # per-chunk precomputed: ekhf blocks [ekh_h0|fk_h0|ekh_h1|fk_h1]
            ekhf = big.tile([128, 16 * 258], F16, name="ekhf")
            vh_all = [big.tile([128, HPC * (D + 1)], F16, name=f"vh{ti}")
                      for ti in range(NT)]
            yt_all = big.tile([128, 2 * T], F16, name="yt_all")

            # ---- phase 1: qkv projection groups ----
            def qk_group(mi, ni):
                tsl = slice(ni * 512, (ni + 1) * 512)
                p_ = bankA()
                for ki in range(NK):
                    nc.tensor.matmul(
                        p_[:, :],
                        kblk(ki, mi - 2) if mi >= 2 else qblk(ki, mi),
                        xt_sb[ki][:, tsl],
                        start=(ki == 0), stop=(ki == NK - 1))
                if mi < 2:
                    nc.vector.tensor_scalar_add(
                        qt_sb[mi][:, tsl], p_[:, :], bqk_sb[mi])
                else:
                    for par in range(2):
                        h = (mi - 2) * 2 + par
                        rs = par * 64          # psum rows holding this head
                        ds = par * 64          # dest rows: k stays in place
                        os = 64 - par * 64     # other rows get k^2
                        nc.scalar.activation(
                            ktsq_sb[h][ds:ds + 64, tsl], p_[rs:rs + 64, :],
                            Ident, bias=bqk_sb[mi][rs:rs + 64, :], scale=1.0)
                        nc.gpsimd.tensor_tensor(
                            ktsq_sb[h][os:os + 64, tsl],
                            ktsq_sb[h][ds:ds + 64, tsl],
                            ktsq_sb[h][ds:ds + 64, tsl], op=Mult)

            # ---- phase 2: exp(omega^T q), exp(omega^T k), row-tiled pairs ----
            def e_q_pair(mi, ni):
                tsl = slice(ni * 512, (ni + 1) * 512)
                banks = []
                for par in range(2):
                    rs = par * 64
                    p_ = bankA()
                    nc.tensor.matmul(p_[:, :], om_sb[rs:rs + 64, :],
                                     qt_sb[mi][rs:rs + 64, tsl],
                                     start=True, stop=True)
                    banks.append(p_)
                for par in range(2):
                    nc.scalar.activation(eq_sb[2 * mi + par][:, tsl],
                                         banks[par][:, :], Exp,
                                         bias=lnsc_sb[:, :], scale=1.0)

            def e_k_pair(pair, ni):
                tsl = slice(ni * 512, (ni + 1) * 512)
                banks = []
                for par in range(2):
                    h, rs = 2 * pair + par, par * 64
                    p_ = bankA()
                    nc.tensor.matmul(p_[:, :], om_sb[rs:rs + 64, :],
                                     ktsq_sb[h][rs:rs + 64, tsl],
                                     start=True, stop=True)
                    banks.append(p_)
                for par in range(2):
                    nc.scalar.activation(ekt_sb[2 * pair + par][:, tsl],
                                         banks[par][:, :], Exp,
                                         bias=lnsc_sb[:, :], scale=1.0)

            def v_group(ti):
                nc.vector.memset(
                    v_sb[ti][:, :].rearrange("p (h c) -> p h c", c=D + 1)
                    [:, :, D:D + 1], 1.0)
                p_ = bankA()
                for ki in range(NK):
                    nc.tensor.matmul(
                        p_[:, 0:HPC * D],
                        xt_sb[ki][:, ti * 128:(ti + 1) * 128],
                        wv_sb[ki][:, :],
                        start=(ki == 0), stop=False)
                nc.tensor.matmul(p_[:, 0:HPC * D], ones_r[:, :], bv_sb[:, :],
                                 start=False, stop=True)
                nc.vector.tensor_copy(
                    v_sb[ti][:, :].rearrange("p (h c) -> p h c", c=D + 1)
                    [:, :, 0:D],
                    p_[:, 0:HPC * D].rearrange("p (h c) -> p h c", c=D))

            # ---- phase 2.5: per-chunk ekh/fk/vh precompute (pipelined) ----
            def chunk_pre(ci, pair):
                h0, h1 = 2 * pair, 2 * pair + 1
                b = pair * NT + ci
                csl = slice(ci * L, (ci + 1) * L)
                pk = ps.tile([128, 512], F32, name="pk", bufs=2)
                nc.tensor.matmul(pk[:, 0:129], ktsq_sb[h0][:, csl],
                                 on_sb[:, :], start=True, stop=True,
                                 skip_group_check=True)
                nc.tensor.matmul(pk[:, 129:258], ktsq_sb[h1][:, csl],
                                 on2_sb[:, :], start=False, stop=True,
                                 skip_group_check=True)
                # exp over [projk|-nsq] for both heads: ekh + fk in one op
                nc.scalar.activation(
                    ekhf[:, b * 258:(b + 1) * 258]
                    .rearrange("p (a c) -> p a c", a=2),
                    pk[:, 0:258].rearrange("p (a c) -> p a c", a=2),
                    Exp, bias=lnsc_sb[:, :], scale=1.0)
                fk0 = ekhf[:, b * 258 + 128:b * 258 + 129]
                fk_b = bass.AP(tensor=fk0.tensor, offset=fk0.offset,
                               ap=[fk0.ap[0], [129, 2], [0, D + 1]])
                nc.vector.scalar_tensor_tensor(
                    vh_all[ci][:, h0 * (D + 1):(h1 + 1) * (D + 1)]
                    .rearrange("p (a c) -> p a c", a=2),
                    v_sb[ci][:, h0 * (D + 1):(h1 + 1) * (D + 1)]
                    .rearrange("p (a c) -> p a c", a=2),
                    VH_SCALE, fk_b, op0=Mult, op1=Mult)

            # ---- phase 3: chunked FAVOR, 3-stage software pipeline ----
            # A(ci): pA matmuls -> atm (vector), state matmuls, spair copy
            # B(ci): inter/intra matmuls into pY -> rc4, ych (vector)
            # C(ci): transposes -> yt copy, c_proj tile, output DMA
            # Emitted as A(c), B(c-1), C(c-2) so every PE op only consumes
            # results produced >= 1 cycle earlier (no PE stalls on vector).
            atm_t = {}
            ych_t = {}
            pyt_t = {}

            def favor_A(ci):
                csl = slice(ci * L, (ci + 1) * L)
                pA = bankA()
                for h in range(HPC):
                    nc.tensor.matmul(pA[:, h * 128:(h + 1) * 128],
                                     ekt_sb[h][:, csl], eq_sb[h][:, csl],
                                     start=(h == 0), stop=True,
                                     skip_group_check=True)
                atm = chk.tile([128, 512], F16, name="atm")
                atm_t[ci] = atm
                mk_b = bass.AP(
                    tensor=mk_sb.tensor, offset=mk_sb.offset,
                    ap=[mk_sb.ap[0], [0, HPC], mk_sb.ap[1]])
                nc.vector.tensor_tensor(
                    atm[:, :].rearrange("p (a c) -> p a c", a=HPC),
                    pA[:, :].rearrange("p (a c) -> p a c", a=HPC),
                    mk_b, op=Mult)
                # state update (bank pre-zeroed: accumulate with start=False)
                for h in range(HPC):
                    b = (h // 2) * NT + ci
                    ssl = h * (D + 1)
                    nc.tensor.matmul(
                        ps_s[:, ssl:ssl + D + 1],
                        ekhf[:, b * 258 + (h % 2) * 129:
                             b * 258 + (h % 2) * 129 + 128],
                        vh_all[ci][:, ssl:ssl + D + 1],
                        start=False, stop=(ci == NT - 1),
                        skip_group_check=True)
                if ci < NT - 1:
                    nc.vector.tensor_copy(sp3[ci % 3][:, :], ps_s[:, :])

            def favor_B(ci):
                csl = slice(ci * L, (ci + 1) * L)
                atm = atm_t.pop(ci)
                pY = bankA()
                for h in range(HPC):
                    ysl = slice(h * (D + 1), (h + 1) * (D + 1))
                    if ci > 0:
                        nc.tensor.matmul(
                            pY[:, ysl], eq_sb[h][:, csl],
                            sp3[(ci - 1) % 3][:, ysl],
                            start=(h == 0), stop=True,
                            skip_group_check=True)
                    nc.tensor.matmul(
                        pY[:, ysl], atm[:, h * 128:(h + 1) * 128],
                        vh_all[ci][:, ysl],
                        start=(ci == 0 and h == 0), stop=True,
                        skip_group_check=True)
                rc4 = col.tile([128, HPC], F32, name="rc4")
                nc.vector.reciprocal(
                    rc4,
                    pY[:, 0:HPC * (D + 1)]
                    .rearrange("p (a c) -> p a c", a=HPC)
                    [:, :, D:D + 1].rearrange("p a c -> p (a c)"))
                ych = chk.tile([128, 256], F16, name="ych")
                ych_t[ci] = ych
                rc_b = bass.AP(
                    tensor=rc4.tensor, offset=rc4.offset,
                    ap=[rc4.ap[0], rc4.ap[1], [0, D]])
                nc.vector.tensor_tensor(
                    ych[:, :].rearrange("p (a c) -> p a c", a=HPC),
                    pY[:, 0:HPC * (D + 1)]
                    .rearrange("p (a c) -> p a c", a=HPC)[:, :, 0:D],
                    rc_b, op=Mult)

            def favor_C1(ci):
                ych = ych_t.pop(ci)
                pyt = ps.tile([128, 256], F16, name="bankA", bufs=5)
                nc.tensor.matmul(pyt[:, 0:128], ych[:, 0:128], id_sb[:, :],
                                 is_transpose=True, start=True, stop=True,
                                 skip_group_check=True)
                nc.tensor.matmul(pyt[:, 128:256], ych[:, 128:256],
                                 id_sb[:, :], is_transpose=True,
                                 start=False, stop=True,
                                 skip_group_check=True)
                nc.vector.tensor_copy(
                    yt_all[:, :].rearrange("p (a t) -> p a t", a=2)
                    [:, :, ci * 128:(ci + 1) * 128],
                    pyt[:, :].rearrange("p (a c) -> p a c", a=2))

            def favor_C2(ci):
                # ---- phase 4 for this token tile ----
                osb = cpy.tile([128, 1024], F16, name="osb")
                for ni in range(2):
                    nsl = slice(ni * 512, (ni + 1) * 512)
                    pp = bankA()
                    for ci2 in range(2):
                        nc.tensor.matmul(
                            pp[:, :],
                            yt_all[:, ci2 * T + ci * 128:
                                   ci2 * T + (ci + 1) * 128],
                            wp_sb[ci2][:, nsl],
                            start=(ci2 == 0), stop=(ci2 == 1))
                    if ni == 1 and ci == NT - 1:
                        nc.vector.tensor_copy(osb[:, nsl], pp[:, :])
                    else:
                        nc.scalar.copy(osb[:, nsl], pp[:, :])
                    nc.sync.dma_start(
                        out=outp[ci * 128:(ci + 1) * 128, nsl],
                        in_=osb[:, nsl])

            # ---- program order ----
            for ni in range(2):
                qk_group(2, ni)
                qk_group(3, ni)
                e_k_pair(0, ni)
                e_k_pair(1, ni)
                for ti in range(4 * ni, 4 * ni + 4):
                    v_group(ti)
                qk_group(0, ni)
                qk_group(1, ni)
                e_q_pair(0, ni)
                e_q_pair(1, ni)
                for ci in range(4 * ni, 4 * ni + 4):
                    chunk_pre(ci, 0)
                    chunk_pre(ci, 1)
            for c in range(NT):
                favor_A(c)
                if c >= 1:
                    favor_B(c - 1)
                if c >= 2:
                    favor_C1(c - 2)
                if c >= 3:
                    favor_C2(c - 3)
            favor_B(NT - 1)
            favor_C1(NT - 2)
            favor_C2(NT - 3)
            favor_C1(NT - 1)
            favor_C2(NT - 2)
            favor_C2(NT - 1)

    _split_waits(nc)
    return nc


_NC_CACHE = None


def _get_nc():
    global _NC_CACHE
    if _NC_CACHE is None:
        _NC_CACHE = build_bass()
    return _NC_CACHE


def _img8(w):
    # [1024, n] -> [128, 8*n] with 128-row blocks laid side by side
    n = w.shape[1]
    return np.ascontiguousarray(
        w.reshape(8, 128, n).transpose(1, 0, 2).reshape(128, 8 * n))


def kernel(x, W_attn, b_attn, W_proj, b_proj, omega):
    from concourse.bass_utils import run_bass_kernel_spmd

    x = np.asarray(x, dtype=np.float32)
    W_attn = np.asarray(W_attn, dtype=np.float32)
    b_attn = np.asarray(b_attn, dtype=np.float32)
    W_proj = np.asarray(W_proj, dtype=np.float32)
    b_proj = np.asarray(b_proj, dtype=np.float32)
    omega = np.asarray(omega, dtype=np.float32)

    B = x.shape[0]
    scale = 1.0 / math.sqrt(D)
    omega2 = np.concatenate([omega, omega], axis=0)
    omnsq = np.zeros((128, 129), np.float32)
    omnsq[0:64, 0:128] = omega
    omnsq[64:128, 128] = -0.5
    omnsq2 = np.zeros((128, 129), np.float32)
    omnsq2[64:128, 0:128] = omega
    omnsq2[0:64, 128] = -0.5
    maskT = np.triu(np.ones((128, 128), np.float32))
    ident = np.eye(128, dtype=np.float32)
    c16_base = np.zeros((128, C16_W), np.float16)
    c16_base[:, C16_OM:C16_OM + 128] = omega2
    c16_base[:, C16_ON:C16_ON + 129] = omnsq
    c16_base[:, C16_ON2:C16_ON2 + 129] = omnsq2
    c16_base[:, C16_MK:C16_MK + 128] = maskT
    c16_base[:, C16_ID:C16_ID + 128] = ident

    # x images: 8 slabs of (ki pair, token half), token half 0 first
    ximgs = []
    for b in range(B):
        xT = np.ascontiguousarray(x[b].T).astype(np.float16)
        Xr = xT.reshape(8, 128, 1024)
        slabs = []
        for j in range(8):
            kp, th = (j % 4) * 2, j // 4
            s = Xr[kp:kp + 2, :, th * 512:(th + 1) * 512]   # [2,128,512]
            slabs.append(s.transpose(1, 0, 2).reshape(128, 1024))
        ximgs.append(np.ascontiguousarray(np.concatenate(slabs, axis=1)))

    in_maps = []
    for core in range(8):
        b, g = core // 4, core % 4
        ch0 = g * HPC * D
        wq = (W_attn[:, ch0:ch0 + HPC * D] * scale).astype(np.float16)
        wk = (W_attn[:, C + ch0:C + ch0 + HPC * D] * scale).astype(np.float16)
        wv_ = W_attn[:, 2 * C + ch0:2 * C + ch0 + HPC * D].astype(np.float16)
        wp_ = W_proj[ch0:ch0 + HPC * D, :].astype(np.float16)
        bqk_ = (np.concatenate([b_attn[ch0:ch0 + HPC * D],
                                b_attn[C + ch0:C + ch0 + HPC * D]]) * scale
                ).astype(np.float32)
        bv_ = b_attn[2 * C + ch0:2 * C + ch0 + HPC * D].astype(np.float16)
        c16 = c16_base.copy()
        c16[0, C16_BV:C16_BV + HPC * D] = bv_
        c32 = bqk_.reshape(4, 128).T.astype(np.float32)
        in_maps.append({
            "ximg": ximgs[b],
            "wqkk": _img8(wk), "wqkq": _img8(wq), "wvimg": _img8(wv_),
            "wpimg": np.ascontiguousarray(
                wp_.reshape(2, 128, 1024).transpose(1, 0, 2)
                .reshape(128, 2048)),
            "consts16": c16, "consts32": np.ascontiguousarray(c32),
        })

    nc = _get_nc()
    res = run_bass_kernel_spmd(nc, in_maps, list(range(8)))

    out = np.zeros((B, T, C), dtype=np.float32)
    for core in range(8):
        out[core // 4] += res.results[core]["outp"]
    out += b_proj[None, None, :]
    return out
